# revision 24
# baseline (speedup 1.0000x reference)
"""Trainium2 Bass kernel for Transformer-XL style relative-position MHA.

Problem shapes (hardcoded): B=8, SEG=512, MEM=512, MODEL=1024, H=16, D=64.
Sharding: pure data-parallel over batch -> core b computes batch element b.

IMPORTANT quirk: the reference splits heads with a FLAT reshape
(torch .view). With projections computed in NATURAL orientation
[token, HD], head h's matrix is the contiguous block rows[32h:32h+32]
(q) / rows[64h:64h+64] (k/v/r) reinterpreted as [S_or_T, 64]. The
per-head d-major layouts (lay[dd, h*T + j]) are built ON-CHIP: PE
transposes of 128x128 blocks of the projection output, then strided
DVE copies (2-byte scatter DMAs are ~50x slower than strided DVE).

r = R @ w_r is batch-independent -> computed once on the HOST; rlay65
(with row 64 = u2 . r_m) is uploaded directly. qlay row 64 = ones;
klay row 64 = u1 . k_j (tiny matmuls) so ac/bd fold the +u terms as a
65th contraction row.

Math per core/head in the [S=512] x [T=1024] index space:
  ac[i,j] = (q_i+u1).k_j ; bd in diagonal coords m = j-i+511 (matmul)
  p = exp(ac/8)*exp(bd/8) (causal mask j<=i+512), normalized; out = p@v
  y = LN(att @ mlp_w + x)*gamma + beta

The circulant shift is applied by writing EB=exp(bd/8) [S,H,T] bf16 to
DRAM (gpsimd/SWDGE queue) and reading it back with a skewed access
pattern (row step H*T-1 elements, sync queue). Out-of-range m spills
into the next head's row; those positions are exactly the causally
masked ones and are zeroed by a triangular band mask on the last 128
columns of each 128-row tile.

DMA queues: sync(SP) = const/act loads + ebs skew reads + attd/x/yout;
gpsimd(SWDGE) = wk/wv/mlpw streams + eb writes + broadcasts. This keeps
both HWDGE+SWDGE rings busy in parallel and off the compute engines.
"""

import functools
import sys

import numpy as np

sys.path.insert(0, "/opt/trn_rl_repo")

import ml_dtypes  # noqa: E402

import concourse.bass as bass  # noqa: E402
import concourse.mybir as mybir  # noqa: E402
import concourse.tile as tile  # noqa: E402

B, SEG, MEM, MODEL, H, D = 8, 512, 512, 1024, 16, 64
TOT = SEG + MEM
HD = H * D
NCORES = 8
IT = SEG // 128                # 4 row tiles of 128 queries
JMAX = [640, 768, 896, 1024]   # per row-tile: columns beyond are fully masked
MMIN = [384, 256, 128, 0]      # per row-tile: smallest rel index m read

F32 = mybir.dt.float32
BF16 = mybir.dt.bfloat16
AF = mybir.ActivationFunctionType
OP = mybir.AluOpType

bf16_np = ml_dtypes.bfloat16


def _emit(tc, t):
    nc = tc.nc
    ctxs = []

    def pool(name, bufs, space="SBUF"):
        p = tc.tile_pool(name=name, bufs=bufs, space=space)
        ctxs.append(p)
        return p.__enter__()

    csts = pool("csts", 1)
    bigp = pool("bigp", 8)        # hts (bf16) -> xs/y/o (f32) [128,1024]
    wpool = pool("wpool", 8)      # streaming weights [128,1024] bf16
    stgp = pool("stgp", 2)        # projection bf16 staging [128,1024]
    qlayp = pool("qlayp", 1)      # qlay [65, 16*512] bf16
    laysp = pool("laysp", 2)      # rlay65 / klay65 / vlayT [<=65, 16*1024] bf16
    vlp = pool("vlp", 8)          # vls [128, 16*64] bf16 x 8 j-tiles
    workp = pool("workp", 10)      # ebs/ea/p/pts/eb [128,<=1024] bf16
    attp = pool("attp", 2)        # attTh -> attP -> attsall
    small = pool("small", 3)
    ps = pool("ps", 2, space="PSUM")      # [128,1024] f32 (2 banks each)
    pst = pool("pst", 2, space="PSUM")    # transpose pairs [128,256] bf16 / [1,512] f32
    psv = pool("psv", 2, space="PSUM")    # pv accum [64,128] f32

    # ---- constants ----
    ident = csts.tile([128, 128], BF16, tag="ident")
    nc.sync.dma_start(ident, t["idm"][:, :])
    tri = csts.tile([128, 128], BF16, tag="tri")
    nc.sync.dma_start(tri, t["trim"][:, :])
    u1l = csts.tile([64, 16], BF16, tag="u1l")
    nc.sync.dma_start(u1l, t["u1l"][:, :])
    masks = csts.tile([128, 4], F32, tag="masks")
    nc.sync.dma_start(masks, t["maskc"][:, :])
    epsb = csts.tile([128, 1], F32, tag="epsb")
    nc.gpsimd.memset(epsb, 1e-5)
    gam = csts.tile([128, MODEL], BF16, tag="gam")
    nc.gpsimd.dma_start(gam, bass.AP(tensor=t["gam"], offset=0, ap=[[0, 128], [1, MODEL]]))
    bet = csts.tile([128, MODEL], BF16, tag="bet")
    nc.gpsimd.dma_start(bet, bass.AP(tensor=t["bet"], offset=0, ap=[[0, 128], [1, MODEL]]))

    ebuf = t["ebuf"]

    # ---- zero strip: rows [0,384) x all heads x m in [0,128) of ebuf ----
    zs = csts.tile([128, 8 * 128], BF16, tag="zs")
    nc.gpsimd.memset(zs, 0.0)
    for blk in range(3):
        for hb in range(2):
            dst = bass.AP(
                tensor=ebuf,
                offset=blk * 128 * H * TOT + hb * 8 * TOT,
                ap=[[H * TOT, 128], [TOT, 8], [1, 128]],
            )
            nc.gpsimd.dma_start(dst, zs.rearrange("p (h m) -> p h m", h=8))

    # ---- rlay65 (host-computed): [65, H*TOT] bf16, row 64 = u2 . r_m ----
    rlay = laysp.tile([65, H * TOT], BF16, tag="lays", name="rlay")
    nc.gpsimd.dma_start(rlay, t["rlay"][:, :])

    # ---- hT (bf16) interleaved with wq so the first matmul starts early ----
    hts = []
    wqs = []
    for mt in range(8):
        ht = bigp.tile([128, TOT], BF16, tag="big", name=f"ht{mt}")
        nc.sync.dma_start(ht, t["hT"][mt * 128:(mt + 1) * 128, :])
        hts.append(ht)
        w = wpool.tile([128, HD], BF16, tag="w", name=f"wq{mt}")
        nc.sync.dma_start(w, t["wq"][mt * 128:(mt + 1) * 128, :])
        wqs.append(w)

    def stream_w(key, eng):
        ws = []
        for mt in range(8):
            w = wpool.tile([128, HD], BF16, tag="w", name=f"{key}{mt}")
            eng.dma_start(w, t[key][mt * 128:(mt + 1) * 128, :])
            ws.append(w)
        return ws

    wks = stream_w("wk", nc.gpsimd)
    wvs = stream_w("wv", nc.gpsimd)

    # ---- natural-orientation projection of token tile -> stg bf16 ----
    def project_tile(ws, tok0, name):
        acc = ps.tile([128, HD], F32, tag="mm", name=f"mm_{name}")
        for c0 in range(0, HD, 512):
            for mt in range(8):
                nc.tensor.matmul(
                    acc[:, c0:c0 + 512],
                    lhsT=hts[mt][:, tok0:tok0 + 128],
                    rhs=ws[mt][:, c0:c0 + 512],
                    start=(mt == 0),
                    stop=(mt == 7),
                )
        stg = stgp.tile([128, HD], BF16, tag="stg", name=f"stg_{name}")
        nc.vector.tensor_copy(stg, acc)
        return stg

    # ---- on-chip lay build: PE transposes + 2 contiguous-run DVE copies ----
    # Lays are stored W-MAJOR: lay[dd, h*span + w*rows + r] where the true
    # key/query index is j = 16r + w. Consumers un-scramble via strided
    # matmul access patterns (free AP [[1, rows-chunk], [rows, 16]]), so the
    # psum->SBUF copies here have contiguous 64/32-elem runs (fast DVE).
    # stg tile tt covers tokens [128*tt, +128); 8 transposes land in one
    # [128, 1024] psum bank; per parity pp=p//64 one [64, 1024] copy.
    def lay_from_stg(stg, tt, lay, rows_per_head, name, wmajor=True):
        la = lay[:, :]
        lstep, loff = la.ap[0][0], la.offset
        span = rows_per_head * 16          # free span of one head (512/1024)
        hper = 128 // rows_per_head        # heads per 128-token tile (4/2)
        rr = rows_per_head                 # tokens/head within tile (64 / 32)
        tpall = pst.tile([128, 1024], BF16, tag="tp", name=f"tp_{name}{tt}")
        for m in range(8):
            nc.tensor.transpose(
                tpall[:, 128 * m:128 * (m + 1)], stg[:, 128 * m:128 * (m + 1)], ident
            )
        sa = tpall[:, :]
        sstep, soff = sa.ap[0][0], sa.offset
        for pp in range(2):
            if wmajor:
                dst_ap = [[lstep, 64], [span, hper], [2 * rr, 8], [1, rr]]
                src_ap = [[sstep, 64], [rr, hper], [128, 8], [1, rr]]
                doff = rr * pp
            else:  # true index-major: lay[dd, h*span + 16*r + w], w = 2m+pp
                dst_ap = [[lstep, 64], [span, hper], [16, rr], [2, 8]]
                src_ap = [[sstep, 64], [rr, hper], [1, rr], [128, 8]]
                doff = pp
            dst = bass.AP(
                tensor=la.tensor,
                offset=loff + (hper * tt) * span + doff,
                ap=dst_ap,
            )
            src = bass.AP(
                tensor=sa.tensor,
                offset=soff + 64 * pp * sstep,
                ap=src_ap,
            )
            nc.vector.tensor_copy(dst, src)

    # ---- strided views: un-scramble w-major lays at the matmul ----
    def q_lhsT(h, it):
        return qlay[:, h * SEG + it * 128:h * SEG + (it + 1) * 128]

    def lay_rhs(lay, nparts, h, j0, n):
        a = lay[:, :]
        return bass.AP(
            tensor=a.tensor, offset=a.offset + h * TOT + j0 // 16,
            ap=[[a.ap[0][0], nparts], [1, n // 16], [64, 16]],
        )

    # ---- q projection -> qlay [65, H*SEG] (row 64 = ones) ----
    qlay = qlayp.tile([65, H * SEG], BF16, tag="qlay")
    nc.vector.memset(qlay[64:65, :], 1.0)
    for tt in range(4):
        stg = project_tile(wqs, SEG + 128 * tt, f"q{tt}")
        lay_from_stg(stg, tt, qlay, 32, "q", wmajor=False)

    # ---- bd scores -> EB=exp(bd/8) -> ebuf (gpsimd writes) ----
    # it-outer/h-inner so eb lands in the order the ac loop consumes it.
    for it in range(IT):
        for h in range(16):
            m0, w_ = MMIN[it], TOT - MMIN[it]
            bd = ps.tile([128, HD], F32, tag="mm", name=f"bd{it}_{h}")
            c0 = 0
            while c0 < w_:
                cw = min(512, w_ - c0)
                nc.tensor.matmul(
                    bd[:, c0:c0 + cw],
                    lhsT=q_lhsT(h, it),
                    rhs=rlay[:, h * TOT + m0 + c0:h * TOT + m0 + c0 + cw],
                    start=True,
                    stop=True,
                )
                c0 += cw
            eb = workp.tile([128, TOT], BF16, tag="wk", name=f"eb{it}_{h}")
            nc.scalar.activation(eb[:, :w_], bd[:, :w_], AF.Exp, scale=0.125)
            dst = bass.AP(
                tensor=ebuf,
                offset=it * 128 * H * TOT + h * TOT + m0,
                ap=[[H * TOT, 128], [1, w_]],
            )
            nc.gpsimd.dma_start(dst, eb[:, :w_])

    # ---- k projection -> klay65 (row 64 = u1 . k_j via matmuls) ----
    klay = laysp.tile([65, H * TOT], BF16, tag="lays", name="klay")
    def urow_tile(tt):
        for h in (2 * tt, 2 * tt + 1):
            for c0 in range(0, TOT, 512):
                up = pst.tile([1, 512], F32, tag="tp", name=f"u{h}_{c0}")
                nc.tensor.matmul(
                    up,
                    lhsT=u1l[:, h:h + 1],
                    rhs=klay[0:64, h * TOT + c0:h * TOT + c0 + 512],
                    start=True,
                    stop=True,
                )
                nc.scalar.copy(klay[64:65, h * TOT + c0:h * TOT + c0 + 512], up)

    for tt in range(8):
        stg = project_tile(wks, 128 * tt, f"k{tt}")
        lay_from_stg(stg, tt, klay, 64, "k", wmajor=False)
        if tt >= 1:
            urow_tile(tt - 1)
    urow_tile(7)

    # ---- v projection -> vlayT [64, H*TOT] -> vls (PE re-transpose) ----
    vlayT = laysp.tile([64, H * TOT], BF16, tag="lays", name="vlayT")
    vls = [
        vlp.tile([128, H * 64], BF16, tag="vl", name=f"vl{jb}") for jb in range(8)
    ]
    def vls_tile(tt):
        for h in (2 * tt, 2 * tt + 1):
            for jb in range(8):
                tp = pst.tile([128, 64], BF16, tag="tp", name=f"vt{h}_{jb}")
                nc.tensor.transpose(
                    tp, vlayT[0:64, h * TOT + jb * 128:h * TOT + (jb + 1) * 128],
                    ident[0:64, 0:64],
                )
                (nc.scalar.copy if jb % 2 else nc.vector.tensor_copy)(
                    vls[jb][:, h * 64:(h + 1) * 64], tp
                )

    for tt in range(8):
        stg = project_tile(wvs, 128 * tt, f"v{tt}")
        lay_from_stg(stg, tt, vlayT, 64, "v", wmajor=False)
        if tt >= 1:
            vls_tile(tt - 1)
    vls_tile(7)

    xs = []
    for it in range(IT):
        x = bigp.tile([128, MODEL], F32, tag="big", name=f"x{it}")
        nc.sync.dma_start(x, t["x_sm"][it * 128:(it + 1) * 128, :])
        xs.append(x)

    # ---- mlp weights (gpsimd ring, after eb writes) ----
    mlps = stream_w("mlpw", nc.gpsimd)

    # ---- scores / softmax / p@v (software-pipelined, skew 2) ----
    # Phase A (iter n): ebs skew read + ac matmul + exp + tri mask.
    # Phase B (iter n): softmax normalize + p transposes + p@v + attTh.
    # Emitting A(n+2) behind B(n) keeps the in-order PE queue fed while
    # iteration n's softmax chain runs on scalar/vector/gpsimd.
    attTh = attp.tile([64, H * SEG], BF16, tag="att", name="attTh")
    attP = attp.tile([64, H * SEG], BF16, tag="att", name="attP")
    iters = [(it, h) for it in range(IT) for h in range(16)]
    astate = {}

    def permute_it(it):
        # attP[dd, cc*512+32h+rr] = attTh[dd, h*512+16rr+cc], rr in it-block
        ao, astep = attTh[:, :].offset, attTh[:, :].ap[0][0]
        po, pstep = attP[:, :].offset, attP[:, :].ap[0][0]
        psrc = bass.AP(
            tensor=attTh[:, :].tensor, offset=ao + 128 * it,
            ap=[[astep, 64], [1, 16], [512, 16], [16, 8]],
        )
        pdst = bass.AP(
            tensor=attP[:, :].tensor, offset=po + 8 * it,
            ap=[[pstep, 64], [512, 16], [32, 16], [1, 8]],
        )
        nc.vector.tensor_copy(pdst, psrc)

    def phase_a(n):
        it, h = iters[n]
        jm = JMAX[it]
        i0 = it * 128
        ebs = workp.tile([128, jm], BF16, tag="wk", name=f"ebs{it}_{h}")
        src = bass.AP(
            tensor=ebuf,
            offset=i0 * H * TOT + h * TOT + (511 - i0),
            ap=[[H * TOT - 1, 128], [1, jm]],
        )
        nc.sync.dma_start(ebs, src)
        acps = ps.tile([128, jm], F32, tag="mm", name=f"ac{it}_{h}")
        c0 = 0
        while c0 < jm:
            cw = min(512, jm - c0)
            nc.tensor.matmul(
                acps[:, c0:c0 + cw],
                lhsT=q_lhsT(h, it),
                rhs=klay[:, h * TOT + c0:h * TOT + c0 + cw],
                start=True,
                stop=True,
            )
            c0 += cw
        ea = workp.tile([128, jm], BF16, tag="wk", name=f"ea{it}_{h}")
        nc.scalar.activation(ea, acps, AF.Exp, scale=0.125)
        nc.gpsimd.tensor_mul(ebs[:, jm - 128:jm], ebs[:, jm - 128:jm], tri)
        astate[n] = (ebs, ea)

    def phase_b(n):
        it, h = iters[n]
        jm = JMAX[it]
        nblk = jm // 128
        i0 = it * 128
        ebs, ea = astate.pop(n)
        p = workp.tile([128, jm], BF16, tag="wk", name=f"p{it}_{h}")
        sums = small.tile([128, 1], F32, tag="sums", name=f"sm{it}_{h}")
        nc.vector.scalar_tensor_tensor(
            out=p, in0=ea, scalar=1.0, in1=ebs,
            op0=OP.mult, op1=OP.mult, accum_out=sums,
        )
        rec = small.tile([128, 1], F32, tag="rec", name=f"rc{it}_{h}")
        nc.vector.reciprocal(rec, sums)
        nc.vector.tensor_scalar(
            out=p, in0=p, scalar1=rec, scalar2=masks[:, it:it + 1],
            op0=OP.mult, op1=OP.mult,
        )
        pts = workp.tile([128, jm], BF16, tag="wk", name=f"pt{it}_{h}")
        cpeng = nc.scalar.copy if n % 8 < 5 else nc.vector.tensor_copy
        jb = 0
        while jb < nblk:
            take = 2 if jb + 1 < nblk else 1
            tp2 = pst.tile([128, 128 * take], BF16, tag="tp", name=f"tp{it}_{h}_{jb}")
            for u in range(take):
                nc.tensor.transpose(
                    tp2[:, 128 * u:128 * (u + 1)],
                    p[:, (jb + u) * 128:(jb + u + 1) * 128],
                    ident,
                )
            cpeng(pts[:, jb * 128:(jb + take) * 128], tp2)
            jb += take
        pv = psv.tile([64, 128], F32, tag="pv", name=f"pv{it}_{h}")
        for jb in range(nblk):
            nc.tensor.matmul(
                pv,
                lhsT=vls[jb][:, 64 * h:64 * h + 64],
                rhs=pts[:, jb * 128:(jb + 1) * 128],
                start=(jb == 0),
                stop=(jb == nblk - 1),
            )
        ceng = nc.scalar.copy if n % 8 >= 5 else nc.vector.tensor_copy
        ceng(attTh[:, h * SEG + i0:h * SEG + i0 + 128], pv)

    phase_a(0)
    phase_a(1)
    phase_a(2)
    for n in range(len(iters)):
        phase_b(n)
        if n + 3 < len(iters):
            phase_a(n + 3)
        if n % 16 == 15:
            permute_it(n // 16)

    # ---- att DRAM hop (permutes already done per-it during the loop) ----
    nc.sync.dma_start(
        bass.AP(tensor=t["attd"], offset=0, ap=[[H * SEG, 64], [1, H * SEG]]),
        attP,
    )
    attsall = attp.tile([128, 8 * SEG], BF16, tag="att", name="attsall")
    for a in range(8):
        for ccp in range(2):
            src = bass.AP(
                tensor=t["attd"],
                offset=(2 * a + ccp) * 512,
                ap=[[H * SEG, 64], [1, 512]],
            )
            (nc.sync.dma_start if a % 2 else nc.scalar.dma_start)(
                attsall[ccp * 64:(ccp + 1) * 64, a * SEG:(a + 1) * SEG], src
            )

    # ---- mlp + residual + layernorm ----
    for it in range(IT):
        acc = ps.tile([128, MODEL], F32, tag="mm", name=f"mlp{it}")
        for half in range(2):
            for dt in range(8):
                nc.tensor.matmul(
                    acc[:, half * 512:(half + 1) * 512],
                    lhsT=attsall[:, dt * SEG + it * 128:dt * SEG + (it + 1) * 128],
                    rhs=mlps[dt][:, half * 512:(half + 1) * 512],
                    start=(dt == 0),
                    stop=(dt == 7),
                )
        y = bigp.tile([128, MODEL], F32, tag="big", name=f"y{it}")
        ysum = small.tile([128, 1], F32, tag="ysum", name=f"ys{it}")
        nc.vector.scalar_tensor_tensor(
            out=y, in0=acc, scalar=1.0, in1=xs[it],
            op0=OP.mult, op1=OP.add, accum_out=ysum,
        )
        sq = ps.tile([128, MODEL], F32, tag="mm", name=f"sq{it}")
        ysq = small.tile([128, 1], F32, tag="ysq", name=f"yq{it}")
        nc.scalar.activation(sq, y, AF.Square, accum_out=ysq)
        mu = small.tile([128, 1], F32, tag="mu", name=f"mu{it}")
        nc.scalar.mul(mu, ysum, 1.0 / MODEL)
        msq = small.tile([128, 1], F32, tag="msq", name=f"mq{it}")
        nc.scalar.mul(msq, ysq, 1.0 / MODEL)
        mu2 = small.tile([128, 1], F32, tag="mu2", name=f"m2{it}")
        nc.vector.tensor_mul(mu2, mu, mu)
        var = small.tile([128, 1], F32, tag="var", name=f"va{it}")
        nc.vector.tensor_tensor(out=var, in0=msq, in1=mu2, op=OP.subtract)
        std = small.tile([128, 1], F32, tag="std", name=f"sd{it}")
        nc.scalar.activation(std, var, AF.Sqrt, bias=epsb)
        rstd = small.tile([128, 1], F32, tag="rstd", name=f"rs{it}")
        nc.vector.reciprocal(rstd, std)
        o = bigp.tile([128, MODEL], F32, tag="big", name=f"o{it}")
        nc.vector.tensor_scalar(
            out=o, in0=y, scalar1=mu, scalar2=rstd,
            op0=OP.subtract, op1=OP.mult,
        )
        nc.vector.tensor_mul(o, o, gam)
        nc.vector.tensor_add(o, o, bet)
        nc.sync.dma_start(t["yout"][it * 128:(it + 1) * 128, :], o)

    for p_ in reversed(ctxs):
        p_.__exit__(None, None, None)


def _split_ctrl_waits(nc, maxw=1):
    """The container's walrus rejects instructions carrying more than 2 sem
    waits ("Too many sync wait commands"). Move excess waits onto preceding
    same-engine NoOps (engines execute their stream in order, so the waits
    still complete before the instruction issues)."""
    n = 0
    for bb in nc.main_func.blocks:
        changed = False
        out = []
        for ins in bb.instructions:
            lim = maxw
            si = ins.sync_info
            if si is not None and si.on_wait and len(si.on_wait) > lim:
                waits = list(si.on_wait)
                while len(waits) > lim:
                    chunk, waits = waits[:lim], waits[lim:]
                    nop = mybir.InstNoOp(
                        name=f"I-wsplit{n}",
                        engine=ins.engine,
                        sync_info=mybir.SyncInfo(on_wait=list(chunk), on_update=[]),
                    )
                    n += 1
                    out.append(nop)
                si.on_wait = waits
                changed = True
            out.append(ins)
        if changed:
            bb.instructions = out


@functools.lru_cache(maxsize=1)
def _build():
    nc = bass.Bass()
    t = {}
    t["hT"] = nc.dram_tensor("hT", [MODEL, TOT], BF16, kind="ExternalInput")
    t["x_sm"] = nc.dram_tensor("x_sm", [SEG, MODEL], F32, kind="ExternalInput")
    for w in ("wq", "wk", "wv"):
        t[w] = nc.dram_tensor(w, [MODEL, HD], BF16, kind="ExternalInput")
    t["mlpw"] = nc.dram_tensor("mlpw", [HD, MODEL], BF16, kind="ExternalInput")
    t["rlay"] = nc.dram_tensor("rlay", [65, H * TOT], BF16, kind="ExternalInput")
    t["u1l"] = nc.dram_tensor("u1l", [64, 16], BF16, kind="ExternalInput")
    t["maskc"] = nc.dram_tensor("maskc", [128, 4], F32, kind="ExternalInput")
    t["gam"] = nc.dram_tensor("gam", [1, MODEL], BF16, kind="ExternalInput")
    t["bet"] = nc.dram_tensor("bet", [1, MODEL], BF16, kind="ExternalInput")
    t["trim"] = nc.dram_tensor("trim", [128, 128], BF16, kind="ExternalInput")
    t["idm"] = nc.dram_tensor("idm", [128, 128], BF16, kind="ExternalInput")
    t["ebuf"] = nc.dram_tensor("ebuf", [SEG, H, TOT], BF16)
    t["attd"] = nc.dram_tensor("attd", [64, H * SEG], BF16)
    t["yout"] = nc.dram_tensor("yout", [SEG, MODEL], F32, kind="ExternalOutput")

    with tile.TileContext(nc) as tc:
        _emit(tc, t)
    _split_ctrl_waits(nc)
    return nc


def _host_inputs(inputs):
    x = np.asarray(inputs["x"], np.float32)
    mem = np.asarray(inputs["mem"], np.float32)
    att_mask = np.asarray(inputs["att_mask"], np.float32)
    u1 = np.asarray(inputs["u1"], np.float32).reshape(H, D)
    u2 = np.asarray(inputs["u2"], np.float32).reshape(H, D)
    R = np.asarray(inputs["R"], np.float32)
    w_r = np.asarray(inputs["w_r"], np.float32)

    h = np.concatenate([mem, x], axis=1)  # [B, TOT, MODEL]

    # rlay65: batch-independent -> host once. r_nat = R[-TOT:] @ w_r;
    # head h = rows [64h:64h+64] flat-viewed as [TOT, 64] (m = 16*rr + ww).
    # rlay[dd, h*TOT + m] = R_head[h, m, dd] (true m-major);
    # row 64 = u2[h] . R_head[h, m].
    rnat = (R[-TOT:] @ w_r).astype(np.float32)
    v4 = rnat.reshape(H, 64, 16, 64)                    # [h, rr, ww, dd]
    body = np.ascontiguousarray(v4.transpose(3, 0, 1, 2)).reshape(64, H * TOT)
    u2row = np.einsum("hrwd,hd->hrw", v4, u2).reshape(1, H * TOT)
    rlay65 = np.concatenate([body, u2row], axis=0).astype(bf16_np)

    shared = {
        "wq": np.asarray(inputs["w_q"], np.float32).astype(bf16_np),
        "wk": np.asarray(inputs["w_k"], np.float32).astype(bf16_np),
        "wv": np.asarray(inputs["w_v"], np.float32).astype(bf16_np),
        "mlpw": np.asarray(inputs["mlp_w"], np.float32).astype(bf16_np),
        "rlay": rlay65,
        "u1l": np.ascontiguousarray(u1.T).astype(bf16_np),
        "gam": np.asarray(inputs["ln_gamma"], np.float32).reshape(1, MODEL).astype(bf16_np),
        "bet": np.asarray(inputs["ln_beta"], np.float32).reshape(1, MODEL).astype(bf16_np),
        "trim": np.tril(np.ones((128, 128), np.float32)).astype(bf16_np),
        "idm": np.eye(128, dtype=np.float32).astype(bf16_np),
    }
    in_maps = []
    for b in range(NCORES):
        m = dict(shared)
        m["hT"] = np.ascontiguousarray(h[b].T).astype(bf16_np)
        m["x_sm"] = np.ascontiguousarray(x[b])
        m["maskc"] = np.ascontiguousarray(att_mask[b].reshape(4, 128).T)
        in_maps.append(m)
    return in_maps


def kernel(**inputs) -> np.ndarray:
    from concourse.bass_utils import run_bass_kernel_spmd

    nc = _build()
    in_maps = _host_inputs(inputs)
    res = run_bass_kernel_spmd(nc, in_maps, list(range(NCORES)))
    out = np.stack([np.asarray(res.results[b]["yout"]) for b in range(NCORES)])
    return out.astype(np.float32)
